# revision 13
# baseline (speedup 1.0000x reference)
"""CapsAlexNet (FLOWER102) forward pass on 8 Trainium2 NeuronCores.

Sharding:
  - conv stack: data-parallel over batch (2 images/core); conv1 via host im2col.
  - capsule routing: capsule dim (i) sharded 8 ways (AllToAll switches from
    batch-sharding to i-sharding); x_hat recomputed per routing pass from SBUF
    via block-diagonal matmuls; AllReduce of [16,102,16] per iteration.
  - FC head: output-sharded (512 cols/core) with AllGather between layers.
"""

import numpy as np
from numpy.lib.stride_tricks import as_strided

import concourse.bass as bass
import concourse.mybir as mybir
import concourse.tile as tile
from concourse import bacc
from concourse.ap import AP
from concourse.bass_utils import run_bass_kernel_spmd

F32 = mybir.dt.float32
AX = mybir.AxisListType
OP = mybir.AluOpType
AF = mybir.ActivationFunctionType

NCORES = 8
B = 16
BC = 2           # images per core
O = 102
D = 16
OD = O * D       # 1632
ITOT = 2592
IPAD = 2688
ILOC = IPAD // NCORES   # 336
G = ILOC // 8           # 42
RG = [list(range(NCORES))]

_CACHE = {}


def _chunks512(total):
    """[0,total) split at 512-float PSUM bank boundaries."""
    out, s = [], 0
    while s < total:
        e = min(s + 512, total)
        out.append((s, e))
        s = e
    return out


def _vp_chunks(h):
    """Global od-slices for the v_p accumulator, aligned to PSUM banks."""
    return [(0, 512), (512, 816)] if h == 0 else \
        [(816, 1024), (1024, 1536), (1536, 1632)]


def _dap(a, offset, dims):
    """Manual AP into a dram-pool tile (which is itself an AP)."""
    return AP(tensor=a.tensor, offset=a.offset + offset,
              ap=[list(d) for d in dims])


def _pv(t, part0, free0, dims):
    """AP into SBUF tile t at (partition part0, free offset free0).

    dims: list of [step, count] for the free dims; partition count inferred
    stays full unless npart given via dims[0] being ('P', count).
    """
    base = t[:]
    fs = base.ap[0][0]          # partition stride == free size
    npart = dims[0][1] if dims[0][0] == "P" else base.ap[0][1]
    rest = dims[1:] if dims[0][0] == "P" else dims
    return AP(tensor=base.tensor, offset=base.offset + part0 * fs + free0,
              ap=[[fs, npart]] + [list(d) for d in rest])


def build_program():
    nc = bacc.Bacc("TRN2", target_bir_lowering=False, debug=False,
                   num_devices=NCORES)

    def din(name, shape):
        return nc.declare_dram_parameter(name, list(shape), F32, isOutput=False)

    xcols = din("xcols", [BC, 363, 2601])
    w1T = din("w1T", [363, 96]); b1c = din("b1c", [96, 1])
    w2T = din("w2T", [25, 96, 256]); b2c = din("b2c", [2, 128, 1])
    wpT = din("wpT", [16, 2, 128, 256]); bpc = din("bpc", [2, 128, 1])
    w3T = din("w3T", [3, 768]); b3c = din("b3c", [2, 128, 1])
    wrg = din("wrg", [G, 64, OD])
    smat = din("smat", [128, 16])
    fw1T = din("fw1T", [144, 128, 512]); fb1r = din("fb1r", [16, 512])
    fw2T = din("fw2T", [32, 128, 512]); fb2r = din("fb2r", [16, 512])
    fw3T = din("fw3T", [32, 128, 102]); fb3r = din("fb3r", [16, 102])
    out_t = nc.declare_dram_parameter("out", [B, O], F32, isOutput=True)

    with tile.TileContext(nc) as tc:
        with tc.tile_pool(name="dram", bufs=1, space="DRAM") as dram:
            _build_body(nc, tc, dram, locals())
    nc.finalize()
    return nc


def _build_body(nc, tc, dram, T):
    xcols, w1T, b1c, w2T, b2c, wpT, bpc = (T["xcols"], T["w1T"], T["b1c"],
                                           T["w2T"], T["b2c"], T["wpT"], T["bpc"])
    w3T, b3c, wrg, smat = T["w3T"], T["b3c"], T["wrg"], T["smat"]
    fw1T, fb1r, fw2T, fb2r, fw3T, fb3r = (T["fw1T"], T["fb1r"], T["fw2T"],
                                          T["fb2r"], T["fw3T"], T["fb3r"])
    out_t = T["out_t"]

    # ---------------- DRAM scratch ----------------
    upc = dram.tile([BC, 20736], F32, tag="upc")
    u_loc = dram.tile([BC, IPAD * 8], F32, tag="uloc")        # [2, 21504]
    u_a2a = dram.tile([NCORES, BC, ILOC * 8], F32, tag="ua2a")  # [8,2,2688]
    u_mine = dram.tile([NCORES, BC, ILOC * 8], F32, tag="umine")
    v_in = [dram.tile([16, OD], F32, tag=f"vin{i}", name=f"vin{i}")
            for i in range(3)]
    v_out = [dram.tile([16, OD], F32, tag=f"vout{i}", name=f"vout{i}")
             for i in range(3)]
    v2d = dram.tile([B * OD], F32, tag="v2d")
    fD2 = dram.tile([18432, B], F32, tag="fD2")
    f1loc = dram.tile([512, B], F32, tag="f1loc")
    f1g = dram.tile([4096, B], F32, tag="f1g")
    f2loc = dram.tile([512, B], F32, tag="f2loc")
    f2g = dram.tile([4096, B], F32, tag="f2g")

    # =========================================================
    # Phase A: conv stack
    # =========================================================
    with (
        tc.tile_pool(name="caw", bufs=1) as cw,
        tc.tile_pool(name="cact", bufs=1) as ca,
        tc.tile_pool(name="cps", bufs=2, space="PSUM") as cps,
        tc.tile_pool(name="cps1", bufs=1, space="PSUM") as cps1,
    ):
        w1t_sb = cw.tile([128, 3 * 96], F32, tag="w1t")
        for kt in range(3):
            rows = 128 if kt < 2 else 107
            nc.sync.dma_start(out=w1t_sb[:rows, kt * 96:(kt + 1) * 96],
                              in_=w1T[kt * 128:kt * 128 + rows, :])
        w2t_sb = cw.tile([96, 25 * 256], F32, tag="w2t")
        nc.sync.dma_start(out=w2t_sb[:].rearrange("p (t o) -> p t o", o=256),
                          in_=w2T.ap().rearrange("t c o -> c t o"))
        wpt_sb = cw.tile([128, 32 * 256], F32, tag="wpt")
        nc.sync.dma_start(
            out=wpt_sb[:].rearrange("p (t k o) -> p t k o", k=2, o=256),
            in_=wpT.ap().rearrange("t k c o -> c t k o"))
        b1_sb = cw.tile([96, 1], F32, tag="b1s")
        nc.sync.dma_start(out=b1_sb[:], in_=b1c[:, :])
        b2_sb = cw.tile([128, 2], F32, tag="b2s")
        nc.sync.dma_start(out=b2_sb[:].rearrange("c (m one) -> c m one", one=1),
                          in_=b2c.ap().rearrange("m c one -> c m one"))
        bp_sb = cw.tile([128, 2], F32, tag="bps")
        nc.sync.dma_start(out=bp_sb[:].rearrange("c (m one) -> c m one", one=1),
                          in_=bpc.ap().rearrange("m c one -> c m one"))

        # ---- conv1 + relu ----
        xc_sb = ca.tile([128, BC * 3 * 2601], F32, tag="xc")
        for img in range(BC):
            for kt in range(3):
                rows = 128 if kt < 2 else 107
                c0 = (img * 3 + kt) * 2601
                nc.sync.dma_start(out=xc_sb[:rows, c0:c0 + 2601],
                                  in_=xcols[img, kt * 128:kt * 128 + rows, :])
        h1_sb = ca.tile([96, BC * 2601], F32, tag="h1")
        for img in range(BC):
            for (n0, n1) in _chunks512(2601):
                ps = cps.tile([96, 512], F32, tag="ps1")
                for kt in range(3):
                    rows = 128 if kt < 2 else 107
                    c0 = (img * 3 + kt) * 2601
                    nc.tensor.matmul(ps[:, :n1 - n0],
                                     w1t_sb[:rows, kt * 96:(kt + 1) * 96],
                                     xc_sb[:rows, c0 + n0:c0 + n1],
                                     start=(kt == 0), stop=(kt == 2))
                nc.scalar.activation(h1_sb[:, img * 2601 + n0:img * 2601 + n1],
                                     ps[:, :n1 - n0], AF.Relu, bias=b1_sb[:, 0:1])

        # ---- maxpool1 -> write into padded conv2 input ----
        p1p_sb = ca.tile([96, BC * 841], F32, tag="p1p")
        nc.vector.memset(p1p_sb[:], 0.0)
        for img in range(BC):
            def h1v(ky, kx):
                return _pv(h1_sb, 0, img * 2601 + ky * 51 + kx,
                           [[102, 25], [2, 25]])
            acc = ca.tile([96, 625], F32, tag="pool1")
            a3 = acc[:].rearrange("p (a b) -> p a b", b=25)
            nc.vector.tensor_max(a3, h1v(0, 0), h1v(0, 1))
            for t in range(2, 9):
                ky, kx = divmod(t, 3)
                nc.vector.tensor_max(a3, a3, h1v(ky, kx))
            dst = _pv(p1p_sb, 0, img * 841 + 2 * 29 + 2, [[29, 25], [1, 25]])
            nc.vector.tensor_copy(out=dst, in_=a3)

        # ---- conv2 + relu ----
        h2_sb = ca.tile([128, 2 * BC * 625], F32, tag="h2")
        for mch in range(2):
            ps2 = {}
            for img in range(BC):
                for nch in range(2):
                    ps2[(img, nch)] = cps1.tile(
                        [128, 512], F32, tag=f"ps2_{img}_{nch}",
                        name=f"ps2_{img}_{nch}")
            for tap in range(25):
                ky, kx = divmod(tap, 5)
                lhs = w2t_sb[:, tap * 256 + mch * 128:tap * 256 + (mch + 1) * 128]
                for img in range(BC):
                    for nch, (oy0, nyy) in enumerate([(0, 13), (13, 12)]):
                        rhs = _pv(p1p_sb, 0,
                                  img * 841 + (oy0 + ky) * 29 + kx,
                                  [[29, nyy], [1, 25]])
                        nc.tensor.matmul(ps2[(img, nch)][:, :nyy * 25], lhs, rhs,
                                         start=(tap == 0), stop=(tap == 24))
            for img in range(BC):
                for nch, (oy0, nyy) in enumerate([(0, 13), (13, 12)]):
                    nc.scalar.activation(
                        h2_sb[:, (mch * BC + img) * 625 + oy0 * 25:
                              (mch * BC + img) * 625 + (oy0 + nyy) * 25],
                        ps2[(img, nch)][:, :nyy * 25], AF.Relu,
                        bias=b2_sb[:, mch:mch + 1])

        # ---- maxpool2 ----
        p2_sb = ca.tile([128, 2 * BC * 144], F32, tag="p2")
        for mch in range(2):
            for img in range(BC):
                base = (mch * BC + img) * 625
                def h2v(ky, kx):
                    return _pv(h2_sb, 0, base + ky * 25 + kx,
                               [[50, 12], [2, 12]])
                dst = p2_sb[:, (mch * BC + img) * 144:(mch * BC + img + 1) * 144]
                d3 = dst.rearrange("p (a b) -> p a b", b=12)
                nc.vector.tensor_max(d3, h2v(0, 0), h2v(0, 1))
                for t in range(2, 9):
                    ky, kx = divmod(t, 3)
                    nc.vector.tensor_max(d3, d3, h2v(ky, kx))

        # ---- primarycaps conv (no relu) ----
        pc_sb = ca.tile([128, 2 * BC * 81], F32, tag="pc")
        for mch in range(2):
            psP = {img: cps1.tile([128, 81], F32, tag=f"psP_{img}",
                                  name=f"psP_{img}")
                   for img in range(BC)}
            for tap in range(16):
                ky, kx = divmod(tap, 4)
                for kc in range(2):
                    lhs = wpt_sb[:, (tap * 2 + kc) * 256 + mch * 128:
                                 (tap * 2 + kc) * 256 + (mch + 1) * 128]
                    for img in range(BC):
                        rhs = _pv(p2_sb, 0, (kc * BC + img) * 144 + ky * 12 + kx,
                                  [[12, 9], [1, 9]])
                        nc.tensor.matmul(psP[img][:], lhs, rhs,
                                         start=(tap == 0 and kc == 0),
                                         stop=(tap == 15 and kc == 1))
            for img in range(BC):
                nc.vector.tensor_scalar_add(
                    pc_sb[:, (mch * BC + img) * 81:(mch * BC + img + 1) * 81],
                    psP[img][:], bp_sb[:, mch:mch + 1])

        for mch in range(2):
            for img in range(BC):
                nc.sync.dma_start(
                    out=upc[img, mch * 128 * 81:(mch + 1) * 128 * 81]
                    .rearrange("(p f) -> p f", f=81),
                    in_=pc_sb[:, (mch * BC + img) * 81:(mch * BC + img + 1) * 81])

        # ---- squash -> u_loc ----
        u_sb = ca.tile([128, BC * 21 * 8], F32, tag="usb")
        nc.vector.memset(u_sb[:], 0.0)
        for img in range(BC):
            nc.sync.dma_start(
                out=u_sb[:, img * 168:img * 168 + 160]
                .rearrange("p (c k) -> p c k", k=8),
                in_=_dap(upc, img * 20736, [[8, 128], [1024, 20], [1, 8]]))
            nc.sync.dma_start(
                out=u_sb[:32, img * 168 + 160:img * 168 + 168],
                in_=_dap(upc, img * 20736 + 20 * 1024, [[8, 32], [1, 8]]))
        n2 = ca.tile([128, BC * 21], F32, tag="sqn2")
        t1 = ca.tile([128, BC * 21], F32, tag="sqt1")
        r1 = ca.tile([128, BC * 21], F32, tag="sqr1")
        sq = ca.tile([128, BC * 168], F32, tag="sqsq")
        nc.scalar.activation(sq[:], u_sb[:], AF.Square)
        nc.vector.tensor_reduce(n2[:], sq[:].rearrange("p (c k) -> p c k", k=8),
                                AX.X, OP.add)
        nc.scalar.add(t1[:], n2[:], 1.0)
        nc.vector.reciprocal(t1[:], t1[:])
        nc.vector.tensor_scalar(t1[:], t1[:], -1.0, 1.0, OP.mult, OP.add)
        nc.vector.tensor_scalar_add(r1[:], n2[:], 1e-8)
        nc.scalar.activation(r1[:], r1[:], AF.Sqrt)
        nc.vector.reciprocal(r1[:], r1[:])
        nc.vector.tensor_mul(t1[:], t1[:], r1[:])
        nc.vector.tensor_mul(
            u_sb[:].rearrange("p (c k) -> p c k", k=8),
            u_sb[:].rearrange("p (c k) -> p c k", k=8),
            t1[:].rearrange("p (c one) -> p c one", one=1)
            .broadcast_to((128, BC * 21, 8)))
        for img in range(BC):
            nc.sync.dma_start(
                out=_dap(u_loc, img * 21504, [[8, 128], [1024, 21], [1, 8]]),
                in_=u_sb[:, img * 168:(img + 1) * 168]
                .rearrange("p (c k) -> p c k", k=8))

    # batch-shard -> i-shard via AllToAll
    nc.sync.dma_start(
        out=_dap(u_a2a, 0, [[5376, NCORES], [2688, BC], [1, 2688]]),
        in_=_dap(u_loc, 0, [[2688, NCORES], [21504, BC], [1, 2688]]))
    nc.gpsimd.collective_compute("AllToAll", OP.bypass, replica_groups=RG,
                                 ins=[u_a2a.opt()], outs=[u_mine.opt()])
    # u_mine as flat [16, 2688]: b-major blocks (core j's 2 images in order)

    # =========================================================
    # Phase B: routing
    # =========================================================
    with (
        tc.tile_pool(name="rt", bufs=1) as rt,
        tc.tile_pool(name="rws", bufs=2) as rws,
        tc.tile_pool(name="rtv", bufs=2) as rtv,
        tc.tile_pool(name="rcx", bufs=2) as rcx,
        tc.tile_pool(name="rsm", bufs=3) as rsm,
    ):
        ubd_T = rt.tile([128, G * 64], F32, tag="ubdT")
        nc.vector.memset(ubd_T[:], 0.0)
        for j in range(8):
            # diag block j as transposed: partitions 16j+b, cols (g, 8j+k)
            nc.sync.dma_start(
                out=_pv(ubd_T, 16 * j, 8 * j, [["P", 16], [64, G], [1, 8]]),
                in_=_dap(u_mine, j * 8, [[2688, 16], [64, G], [1, 8]]))
        ident = rt.tile([128, 128], F32, tag="ident")
        from concourse.masks import make_identity
        make_identity(nc, ident[:])
        ubd_sb = rt.tile([64, G * 128], F32, tag="ubd")
        with tc.tile_pool(name="rtp", bufs=2, space="PSUM") as rtp:
            for g in range(G):
                tp = rtp.tile([64, 128], F32, tag="ptp")
                nc.tensor.transpose(tp[:], ubd_T[:, g * 64:(g + 1) * 64],
                                    ident[:])
                nc.scalar.copy(ubd_sb[:, g * 128:(g + 1) * 128], tp[:])
        rpx_cm = tc.tile_pool(name="rpx", bufs=2, space="PSUM")
        rpv_cm = tc.tile_pool(name="rpv", bufs=1, space="PSUM")
        rpx = rpx_cm.__enter__()
        rpv = rpv_cm.__enter__()
        smat_sb = rt.tile([128, 16], F32, tag="smt")
        nc.sync.dma_start(out=smat_sb[:], in_=smat.ap())
        b_sb = rt.tile([128, G * O], F32, tag="blog")
        vrep_sb = rt.tile([128, OD], F32, tag="vrep")
        v_sb = rt.tile([16, OD], F32, tag="vsb")
        vsum_sb = rt.tile([16, OD], F32, tag="vsum")

        def squash16(src, dst):
            qn2 = rsm.tile([16, O], F32, tag="q16a")
            qt = rsm.tile([16, O], F32, tag="q16b")
            qr = rsm.tile([16, O], F32, tag="q16c")
            qs = rsm.tile([16, OD], F32, tag="q16d")
            nc.scalar.activation(qs[:], src, AF.Square)
            nc.vector.tensor_reduce(qn2[:],
                                    qs[:].rearrange("p (o d) -> p o d", d=D),
                                    AX.X, OP.add)
            nc.scalar.add(qt[:], qn2[:], 1.0)
            nc.vector.reciprocal(qt[:], qt[:])
            nc.vector.tensor_scalar(qt[:], qt[:], -1.0, 1.0, OP.mult, OP.add)
            nc.vector.tensor_scalar_add(qr[:], qn2[:], 1e-8)
            nc.scalar.activation(qr[:], qr[:], AF.Sqrt)
            nc.vector.reciprocal(qr[:], qr[:])
            nc.vector.tensor_mul(qt[:], qt[:], qr[:])
            nc.vector.tensor_mul(
                dst.rearrange("p (o d) -> p o d", d=D),
                src.rearrange("p (o d) -> p o d", d=D),
                qt[:].rearrange("p (o one) -> p o one", one=1)
                .broadcast_to((16, O, D)))

        def vrep_fill():
            for j in range(8):
                nc.sync.dma_start(out=vrep_sb[j * 16:(j + 1) * 16, :],
                                  in_=v_sb[:])

        # ---- pass 0 (uniform c: v0 = squash(sum_i x_hat / 102)) ----
        pvp = rpv.tile([16, 2048], F32, tag="pvp")
        for g in range(G):
            wt = rws.tile([64, OD], F32, tag="wt")
            nc.sync.dma_start(out=wt[:], in_=wrg[g, :, :])
            lhs = ubd_sb[:, g * 128:(g + 1) * 128]
            for h in range(2):
                X = rpx.tile([128, 1024], F32, tag="px")
                for (c0, c1) in _chunks512(816):
                    nc.tensor.matmul(X[:, c0:c1], lhs,
                                     wt[:, h * 816 + c0:h * 816 + c1],
                                     start=True, stop=True)
                cx = rcx.tile([128, 816], F32, tag="cx")
                nc.scalar.copy(cx[:], X[:, 0:816])
                for (c0, c1) in _vp_chunks(h):
                    nc.tensor.matmul(pvp[:, c0:c1], smat_sb[:],
                                     cx[:, c0 - h * 816:c1 - h * 816],
                                     start=(g == 0), stop=(g == G - 1),
                                     skip_group_check=True)
        vps = rsm.tile([16, OD], F32, tag="vps")
        nc.scalar.mul(vps[:], pvp[:, 0:OD], 1.0 / O)
        nc.sync.dma_start(out=v_in[0], in_=vps[:])
        nc.gpsimd.collective_compute("AllReduce", OP.add, replica_groups=RG,
                                     ins=[v_in[0].opt()], outs=[v_out[0].opt()])
        nc.sync.dma_start(out=vsum_sb[:], in_=v_out[0])
        squash16(vsum_sb[:], v_sb[:])
        vrep_fill()

        # ---- passes 1,2 ----
        for it in (1, 2):
            pvp = rpv.tile([16, 2048], F32, tag="pvp")
            for g in range(G):
                wt = rws.tile([64, OD], F32, tag="wt")
                nc.sync.dma_start(out=wt[:], in_=wrg[g, :, :])
                lhs = ubd_sb[:, g * 128:(g + 1) * 128]
                Xh = []
                for h in range(2):
                    X = rpx.tile([128, 1024], F32, tag="px")
                    Xh.append(X)
                    for (c0, c1) in _chunks512(816):
                        nc.tensor.matmul(X[:, c0:c1], lhs,
                                         wt[:, h * 816 + c0:h * 816 + c1],
                                         start=True, stop=True)
                    tv = rtv.tile([128, 816], F32, tag="tv")
                    nc.vector.tensor_mul(tv[:], X[:, 0:816],
                                         vrep_sb[:, h * 816:(h + 1) * 816])
                    bsl = b_sb[:, g * O + h * 51:g * O + (h + 1) * 51]
                    if it == 1:
                        nc.vector.tensor_reduce(
                            bsl, tv[:].rearrange("p (o d) -> p o d", d=D),
                            AX.X, OP.add)
                    else:
                        tr = rtv.tile([128, 51], F32, tag="tr")
                        nc.vector.tensor_reduce(
                            tr[:], tv[:].rearrange("p (o d) -> p o d", d=D),
                            AX.X, OP.add)
                        nc.vector.tensor_add(bsl, bsl, tr[:])
                bsl_g = b_sb[:, g * O:(g + 1) * O]
                negm = rsm.tile([128, 1], F32, tag="negm")
                nc.vector.tensor_reduce(negm[:], bsl_g, AX.X, OP.max,
                                        negate=True)
                e = rsm.tile([128, O], F32, tag="e")
                s = rsm.tile([128, 1], F32, tag="s")
                nc.scalar.activation(e[:], bsl_g, AF.Exp, bias=negm[:, 0:1],
                                     accum_out=s[:, 0:1])
                rs = rsm.tile([128, 1], F32, tag="rs")
                nc.vector.reciprocal(rs[:], s[:])
                for h in range(2):
                    cx = rcx.tile([128, 816], F32, tag="cx")
                    eb = e[:, h * 51:(h + 1) * 51] \
                        .rearrange("p (o one) -> p o one", one=1) \
                        .broadcast_to((128, 51, D))
                    nc.vector.scalar_tensor_tensor(
                        out=cx[:].rearrange("p (o d) -> p o d", d=D),
                        in0=Xh[h][:, 0:816].rearrange("p (o d) -> p o d", d=D),
                        scalar=rs[:, 0:1], in1=eb, op0=OP.mult, op1=OP.mult)
                    for (c0, c1) in _vp_chunks(h):
                        nc.tensor.matmul(pvp[:, c0:c1], smat_sb[:],
                                         cx[:, c0 - h * 816:c1 - h * 816],
                                         start=(g == 0), stop=(g == G - 1),
                                         skip_group_check=True)
            vps = rsm.tile([16, OD], F32, tag="vps")
            nc.scalar.copy(vps[:], pvp[:, 0:OD])
            nc.sync.dma_start(out=v_in[it], in_=vps[:])
            nc.gpsimd.collective_compute(
                "AllReduce", OP.add, replica_groups=RG,
                ins=[v_in[it].opt()], outs=[v_out[it].opt()])
            nc.sync.dma_start(out=vsum_sb[:], in_=v_out[it])
            squash16(vsum_sb[:], v_sb[:])
            if it == 1:
                vrep_fill()

        nc.sync.dma_start(out=v2d.rearrange("(p f) -> p f", f=OD),
                          in_=v_sb[:])
        rpv_cm.__exit__(None, None, None)
        rpx_cm.__exit__(None, None, None)

    # =========================================================
    # Phase C: caps conv + FC head
    # =========================================================
    with (
        tc.tile_pool(name="fcw", bufs=1) as fcw,
        tc.tile_pool(name="fcs", bufs=3) as fcs,
        tc.tile_pool(name="fca", bufs=1) as fca,
        tc.tile_pool(name="fps", bufs=2, space="PSUM") as fps,
    ):
        caps3 = fca.tile([3, B * OD], F32, tag="caps3")
        for kh in range(3):
            ln = B * OD - kh * D
            nc.sync.dma_start(
                out=caps3[kh:kh + 1, 0:ln],
                in_=v2d[kh * D:kh * D + ln].rearrange("(one f) -> one f", one=1))
        w3t_sb = fcw.tile([3, 768], F32, tag="w3t")
        nc.sync.dma_start(out=w3t_sb[:], in_=w3T.ap())
        b3_sb = fcw.tile([128, 2], F32, tag="b3s")
        nc.sync.dma_start(out=b3_sb[:].rearrange("c (m one) -> c m one", one=1),
                          in_=b3c.ap().rearrange("m c one -> c m one"))
        h3_sb = fca.tile([128, 2 * B * 350], F32, tag="h3")
        for mch in range(2):
            for b in range(B):
                ps = fps.tile([128, 512], F32, tag="ps3")
                for kw in range(3):
                    rhs = _pv(caps3, 0, b * OD + kw,
                              [["P", 3], [32, 50], [2, 7]])
                    nc.tensor.matmul(
                        ps[:, :350],
                        w3t_sb[:, (kw * 2 + mch) * 128:(kw * 2 + mch + 1) * 128],
                        rhs, start=(kw == 0), stop=(kw == 2))
                nc.scalar.activation(
                    h3_sb[:, mch * B * 350 + b * 350:mch * B * 350 + (b + 1) * 350],
                    ps[:, :350], AF.Relu, bias=b3_sb[:, mch:mch + 1])
        p3_sb = fca.tile([128, 2 * B * 72], F32, tag="p3")
        for mch in range(2):
            def h3v(ky, kx):
                return _pv(h3_sb, 0, mch * B * 350 + ky * 7 + kx,
                           [[350, B], [14, 24], [2, 3]])
            # pos-major output: col = (oy*3+ox)*16 + b
            dst = _pv(p3_sb, 0, mch * B * 72, [[1, B], [48, 24], [16, 3]])
            nc.vector.tensor_max(dst, h3v(0, 0), h3v(0, 1))
            for t in range(2, 9):
                ky, kx = divmod(t, 3)
                nc.vector.tensor_max(dst, dst, h3v(ky, kx))
        for mch in range(2):
            nc.sync.dma_start(
                out=_dap(fD2, mch * 128 * 72 * B,
                         [[72 * B, 128], [B, 72], [1, B]]),
                in_=_pv(p3_sb, 0, mch * B * 72, [[16, 72], [1, B]]))
        f_sb = fca.tile([128, 144 * B], F32, tag="fsb")
        nc.sync.dma_start(out=f_sb[:].rearrange("p (t b) -> p t b", b=B),
                          in_=_dap(fD2, 0, [[B, 128], [128 * B, 144], [1, B]]))

        def fc_layer(lhs_sb, n_kt, wstream, n_out, fbias, relu, fout_dram,
                     idx):
            psf = fps.tile([16, 512], F32, tag="psf")
            for kt in range(n_kt):
                fwt = fcs.tile([128, n_out], F32, tag=f"fwt{idx}")
                nc.sync.dma_start(out=fwt[:], in_=wstream[kt, :, :])
                nc.tensor.matmul(psf[:, :n_out],
                                 lhs_sb[:, kt * B:(kt + 1) * B], fwt[:],
                                 start=(kt == 0), stop=(kt == n_kt - 1))
            fb_sb = fcw.tile([16, n_out], F32, tag=f"fb{idx}")
            nc.sync.dma_start(out=fb_sb[:], in_=fbias.ap())
            res = fca.tile([16, n_out], F32, tag=f"fr{idx}")
            nc.vector.tensor_add(res[:], psf[:, :n_out], fb_sb[:])
            if relu:
                nc.scalar.activation(res[:], res[:], AF.Relu)
            if fout_dram is not None:
                nc.sync.dma_start(
                    out=_dap(fout_dram, 0, [[1, 16], [16, n_out]]),
                    in_=res[:])
            return res

        fc_layer(f_sb, 144, fw1T, 512, fb1r, True, f1loc, 1)
        nc.gpsimd.collective_compute("AllGather", OP.bypass, replica_groups=RG,
                                     ins=[f1loc.opt()], outs=[f1g.opt()])
        f2_sb = fca.tile([128, 32 * B], F32, tag="f2sb")
        nc.sync.dma_start(out=f2_sb[:].rearrange("p (t b) -> p t b", b=B),
                          in_=_dap(f1g, 0, [[B, 128], [128 * B, 32], [1, B]]))
        fc_layer(f2_sb, 32, fw2T, 512, fb2r, True, f2loc, 2)
        nc.gpsimd.collective_compute("AllGather", OP.bypass, replica_groups=RG,
                                     ins=[f2loc.opt()], outs=[f2g.opt()])
        f3_sb = fca.tile([128, 32 * B], F32, tag="f3sb")
        nc.sync.dma_start(out=f3_sb[:].rearrange("p (t b) -> p t b", b=B),
                          in_=_dap(f2g, 0, [[B, 128], [128 * B, 32], [1, B]]))
        res3 = fc_layer(f3_sb, 32, fw3T, 102, fb3r, False, None, 3)
        nc.sync.dma_start(out=out_t[:, :], in_=res3[:])


def _prep_inputs(inputs):
    x = np.ascontiguousarray(inputs["x"], dtype=np.float32)
    w1, b1 = inputs["w1"], inputs["b1"]
    w2, b2 = inputs["w2"], inputs["b2"]
    wp, bp = inputs["wp"], inputs["bp"]
    Wcap = inputs["Wcap"]
    w3, b3 = inputs["w3"], inputs["b3"]
    fw1, fb1 = inputs["fw1"], inputs["fb1"]
    fw2, fb2 = inputs["fw2"], inputs["fb2"]
    fw3, fb3 = inputs["fw3"], inputs["fb3"]

    s = x.strides
    xw = as_strided(x, shape=(B, 3, 11, 11, 51, 51),
                    strides=(s[0], s[1], s[2], s[3], 4 * s[2], 4 * s[3]))
    xcols = np.ascontiguousarray(xw).reshape(B, 363, 2601)

    w1T = np.ascontiguousarray(np.asarray(w1).reshape(96, 363).T)
    w2T = np.ascontiguousarray(np.asarray(w2).transpose(2, 3, 1, 0)
                               ).reshape(25, 96, 256)
    wpT = np.ascontiguousarray(np.asarray(wp).transpose(2, 3, 1, 0)
                               ).reshape(16, 2, 128, 256)
    w3T = np.ascontiguousarray(
        np.asarray(w3).reshape(256, 9).T.reshape(3, 3, 256)).reshape(3, 768)
    # w3T[kh, kw*256 + oc]  -> but kernel slices (kw*2+mch)*128: same layout.

    Wp = np.zeros((O, IPAD, D, 8), np.float32)
    Wp[:, :ITOT] = np.asarray(Wcap)
    wrg_all = np.ascontiguousarray(
        Wp.reshape(O, NCORES, G, 8, D, 8).transpose(1, 2, 3, 5, 0, 4)
    ).reshape(NCORES, G, 64, OD)

    fw1 = np.asarray(fw1); fw2 = np.asarray(fw2); fw3 = np.asarray(fw3)
    fw1T_all = np.ascontiguousarray(
        fw1.reshape(NCORES, 512, 18432).transpose(0, 2, 1)
    ).reshape(NCORES, 144, 128, 512)
    fw2T_all = np.ascontiguousarray(
        fw2.reshape(NCORES, 512, 4096).transpose(0, 2, 1)
    ).reshape(NCORES, 32, 128, 512)
    fw3T = np.ascontiguousarray(fw3.T).reshape(32, 128, 102)

    shared = dict(
        w1T=w1T, b1c=np.asarray(b1).reshape(96, 1),
        w2T=w2T, b2c=np.asarray(b2).reshape(2, 128, 1),
        wpT=wpT, bpc=np.asarray(bp).reshape(2, 128, 1),
        w3T=w3T, b3c=np.asarray(b3).reshape(2, 128, 1),
        smat=np.tile(np.eye(16, dtype=np.float32), (8, 1)),
        fw3T=fw3T,
        fb3r=np.tile(np.asarray(fb3).reshape(1, 102), (16, 1)))
    in_maps = []
    for r in range(NCORES):
        m = dict(shared)
        m["xcols"] = xcols[2 * r:2 * r + 2]
        m["wrg"] = wrg_all[r]
        m["fw1T"] = fw1T_all[r]
        m["fw2T"] = fw2T_all[r]
        m["fb1r"] = np.tile(np.asarray(fb1)[512 * r:512 * (r + 1)].reshape(1, 512),
                            (16, 1))
        m["fb2r"] = np.tile(np.asarray(fb2)[512 * r:512 * (r + 1)].reshape(1, 512),
                            (16, 1))
        in_maps.append({k: np.ascontiguousarray(v, dtype=np.float32)
                        for k, v in m.items()})
    return in_maps


def kernel(**inputs):
    if "nc" not in _CACHE:
        _CACHE["nc"] = build_program()
    in_maps = _prep_inputs(inputs)
    res = run_bass_kernel_spmd(_CACHE["nc"], in_maps, list(range(NCORES)))
    _CACHE["last_exec_ns"] = res.exec_time_ns
    return np.asarray(res.results[0]["out"], dtype=np.float32)
